# revision 23
# baseline (speedup 1.0000x reference)
import math

import numpy as np

import concourse.bacc as bacc
import concourse.mybir as mybir
import concourse.tile as tile
from concourse import bass_utils

N = 100000
H = 256
NCORES = 8
NPER = N // NCORES
P = 128
TILES = 98
NPAD = P * TILES
BLK = 14
NBLK = TILES // BLK
NBANK = 4
SHIFT = 8.0
NEG = np.float32(-1e9)

NB_HALF = 2
NACT_HALF = 2
NBT = NBLK * 2 * NB_HALF

TRACE_OPTS: dict = {}
LAST_RESULTS = None
LAST_INTERNALS: dict = {}

_prog_cache: dict = {}


def _build_program():
    key = "v9"
    if key in _prog_cache:
        return _prog_cache[key]

    f32 = mybir.dt.float32
    f16 = mybir.dt.float16
    nc = bacc.Bacc(
        "TRN2",
        target_bir_lowering=False,
        debug=False,
        enable_asserts=False,
        num_devices=NCORES,
    )
    hh = nc.dram_tensor("hh", [NPAD, 2 * H], f16, kind="ExternalInput").ap()
    hb = nc.dram_tensor("hb", [2 * H, NBT * P], f16, kind="ExternalInput").ap()
    ub = nc.dram_tensor("ub", [P, 2 * H], f16, kind="ExternalInput").ap()
    uc = nc.dram_tensor("uc", [P, 4], f16, kind="ExternalInput").ap()
    mg = nc.dram_tensor("mg", [P, TILES], f32, kind="ExternalInput").ap()
    t_out = nc.dram_tensor("t_out", [NBANK, 2 * H], f32, kind="ExternalOutput").ap()
    s_out = nc.dram_tensor("s_out", [P, 1], f32, kind="ExternalOutput").ap()

    hh_g = hh.rearrange("(p t) h -> p t h", t=TILES)
    hb_g = hb.rearrange("(c p) n -> p c n", p=P)

    HB = BLK // 2
    nA = HB - NB_HALF

    with tile.TileContext(nc) as tc:
        with (
            tc.tile_pool(name="singles", bufs=1) as singles,
            tc.tile_pool(name="blocks", bufs=5) as blocks,
            tc.tile_pool(name="small", bufs=4) as small,
            tc.tile_pool(name="scratch", bufs=3) as scratch,
            tc.tile_pool(name="psum", bufs=1, space="PSUM") as psum,
        ):
            u_sb = singles.tile([P, 2 * H], f16)
            nc.sync.dma_start(out=u_sb, in_=ub)
            uc_sb = singles.tile([P, 4], f16)
            nc.sync.dma_start(out=uc_sb, in_=uc)
            m_sb = singles.tile([P, TILES], f32)
            nc.sync.dma_start(out=m_sb, in_=mg)
            p_grid = singles.tile([P, TILES], f16)
            t_ps = []
            for j in range(NBANK):
                tpsj = psum.tile([1, 2 * H], f32, tag=f"tps{j}")
                t_ps.append(tpsj)
            c_ps = []
            for j in range(3):
                cpsj = psum.tile([P, 1], f32, tag=f"cps{j}")
                c_ps.append(cpsj)

            for b in range(NBLK):
                t0 = b * BLK
                buf = blocks.tile([P, BLK, 2 * H], f16)
                nc.sync.dma_start(out=buf, in_=hh_g[:, t0:t0 + BLK, :])
                tb = blocks.tile([P, 4, 2 * NB_HALF * P], f16, tag="tb")
                nc.sync.dma_start(
                    out=tb,
                    in_=hb_g[:, :, b * 2 * NB_HALF * P:(b + 1) * 2 * NB_HALF * P],
                )
                cblk = small.tile([P, BLK], f32)
                for half in range(2):
                    g0 = half * HB
                    n_act = 2 if (2 * b + half) % 2 == 0 else 3
                    for idx in range(nA):
                        g = g0 + idx
                        c = t0 + g
                        if idx < n_act:
                            scv = scratch.tile([P, 2 * H], f16, tag="dveout")
                            nc.vector.tensor_mul(scv, buf[:, g, :], u_sb)
                            sc2 = scratch.tile([P, 2 * H], f16, tag="actout")
                            nc.scalar.activation(
                                out=sc2,
                                in_=scv,
                                func=mybir.ActivationFunctionType.Identity,
                                bias=0.0,
                                scale=1.0,
                                accum_out=cblk[:, g:g + 1],
                            )
                        else:
                            sc = scratch.tile([P, 2 * H], f16, tag="sttout")
                            nc.vector.scalar_tensor_tensor(
                                out=sc,
                                in0=buf[:, g, :],
                                scalar=1.0,
                                in1=u_sb,
                                op0=mybir.AluOpType.mult,
                                op1=mybir.AluOpType.mult,
                                accum_out=cblk[:, g:g + 1],
                            )
                    for j in range(NB_HALF):
                        g = g0 + nA + j
                        c = t0 + g
                        k = (2 * half + j) * P
                        bank = c % 3
                        for ch in range(4):
                            nc.tensor.matmul(
                                c_ps[bank],
                                lhsT=tb[:, ch, k:k + P],
                                rhs=uc_sb[:, ch:ch + 1],
                                start=(ch == 0),
                                stop=(ch == 3),
                            )
                        nc.scalar.activation(
                            out=p_grid[:, c:c + 1],
                            in_=c_ps[bank],
                            func=mybir.ActivationFunctionType.Exp,
                            bias=m_sb[:, c:c + 1],
                            scale=1.0,
                        )
                    cb2 = small.tile([P, nA], f32, tag="cb2")
                    nc.vector.tensor_add(
                        cb2, cblk[:, g0:g0 + nA], m_sb[:, t0 + g0:t0 + g0 + nA]
                    )
                    nc.scalar.activation(
                        out=p_grid[:, t0 + g0:t0 + g0 + nA],
                        in_=cb2,
                        func=mybir.ActivationFunctionType.Exp,
                        bias=0.0,
                        scale=1.0,
                    )
                for g in range(BLK):
                    c = t0 + g
                    nc.tensor.matmul(
                        t_ps[c % NBANK],
                        lhsT=p_grid[:, c:c + 1],
                        rhs=buf[:, g, :],
                        start=(c < NBANK),
                        stop=(c >= TILES - NBANK),
                    )

            for j in range(NBANK):
                t_sb = small.tile([1, 2 * H], f32, tag="tsb")
                nc.vector.tensor_copy(t_sb, t_ps[j])
                nc.sync.dma_start(out=t_out[j:j + 1, :], in_=t_sb)
            s_col = singles.tile([P, 1], f32)
            nc.vector.reduce_sum(out=s_col, in_=p_grid, axis=mybir.AxisListType.X)
            nc.sync.dma_start(out=s_out, in_=s_col)

    nc.compile()
    _prog_cache[key] = nc
    return nc


def _run_device(h_static, h_dynamic, u_cat, mask_bias):
    global LAST_RESULTS
    nc = _build_program()

    u16 = u_cat.astype(np.float16)
    u_bcast = np.ascontiguousarray(np.broadcast_to(u16, (P, 2 * H)))
    u_colT = np.ascontiguousarray(u16.reshape(4, P).T)

    HB = BLK // 2
    nA = HB - NB_HALF

    in_maps = []
    for c in range(NCORES):
        lo = c * NPER
        h16 = np.zeros((NPAD, 2 * H), np.float16)
        h16[:NPER, 0:H] = h_static[lo:lo + NPER]
        h16[:NPER, H:2 * H] = h_dynamic[lo:lo + NPER]
        mb = np.concatenate(
            [mask_bias[lo:lo + NPER], np.full(NPAD - NPER, NEG, np.float32)]
        )
        grid = np.ascontiguousarray(mb.reshape(P, TILES))
        hbt = np.empty((2 * H, NBT * P), np.float16)
        kk = 0
        for b in range(NBLK):
            for half in range(2):
                for j in range(NB_HALF):
                    t = b * BLK + half * HB + nA + j
                    hbt[:, kk * P:(kk + 1) * P] = h16[t::TILES, :].T
                    kk += 1
        in_maps.append({"hh": h16, "hb": np.ascontiguousarray(hbt),
                        "ub": u_bcast, "uc": u_colT, "mg": grid})

    res = bass_utils.run_bass_kernel_spmd(
        nc, in_maps, core_ids=list(range(NCORES)), **TRACE_OPTS
    )
    LAST_RESULTS = res

    t = np.zeros(2 * H, np.float64)
    s = 0.0
    for c in range(NCORES):
        t += res.results[c]["t_out"].astype(np.float64).sum(axis=0)
        s += float(res.results[c]["s_out"].astype(np.float64).sum())
    return t, s


def kernel(
    h_dynamic,
    h_static,
    W_static_kvl,
    W_dyn_kvl,
    W_q,
    W1,
    b1,
    W2,
    b2,
    valid_mask,
    current_node,
):
    h_dynamic = np.asarray(h_dynamic, np.float32)
    h_static = np.asarray(h_static, np.float32)
    W_static_kvl = np.asarray(W_static_kvl, np.float32)
    W_dyn_kvl = np.asarray(W_dyn_kvl, np.float32)
    W_q = np.asarray(W_q, np.float32)
    W1 = np.asarray(W1, np.float32)
    b1 = np.asarray(b1, np.float32)
    W2 = np.asarray(W2, np.float32)
    b2 = np.asarray(b2, np.float32)
    valid = np.asarray(valid_mask).astype(bool)
    cur = int(current_node)

    scale = 1.0 / math.sqrt(H)

    h_cur = (h_static[cur].astype(np.float64) + h_dynamic[cur].astype(np.float64))
    q = h_cur @ W_q.astype(np.float64)
    u_s = (W_static_kvl[:, 0:H].astype(np.float64) @ q) * scale
    u_d = (W_dyn_kvl[:, 0:H].astype(np.float64) @ q) * scale
    u_cat = np.concatenate([u_s, u_d]).astype(np.float32)

    mask_bias = np.where(valid, np.float32(-SHIFT), NEG).astype(np.float32)

    t, s = _run_device(h_static, h_dynamic, u_cat, mask_bias)

    W_vs = W_static_kvl[:, H:2 * H].astype(np.float64)
    W_vd = W_dyn_kvl[:, H:2 * H].astype(np.float64)
    context = (t[:H] @ W_vs + t[H:] @ W_vd) / s

    fuse = np.concatenate([h_cur, context])
    hidden = np.maximum(fuse @ W1.astype(np.float64) + b1.astype(np.float64), 0.0)
    logit = float(hidden @ W2.astype(np.float64)[:, 0] + float(b2[0]))

    logits_all = np.where(valid, np.float32(logit), NEG).astype(np.float32)

    LAST_INTERNALS.update(
        dict(u_cat=u_cat, t=t, s=s, context=context, logit=logit)
    )

    import jax
    import jax.numpy as jnp

    cpu = jax.devices("cpu")[0]
    with jax.default_device(cpu):
        logits_j = jnp.asarray(logits_all)
        choice = jax.random.categorical(jax.random.key(1), logits_j)
        log_probs = jax.nn.log_softmax(logits_j)
        log_prob = log_probs[choice]
        choice_np = np.asarray(choice)
        log_prob_np = np.asarray(log_prob)

    return (choice_np, log_prob_np)


# revision 24
# speedup vs baseline: 1.0379x; 1.0379x over previous
import math

import numpy as np

import concourse.bacc as bacc
import concourse.mybir as mybir
import concourse.tile as tile
from concourse import bass_utils

N = 100000
H = 256
NCORES = 8
NPER = N // NCORES
P = 128
TILES = 98
NPAD = P * TILES
BLK = 14
NBLK = TILES // BLK
NBANK = 4
SHIFT = 8.0
NEG = np.float32(-1e9)

NB_HALF = 2
NACT_HALF = 2
NBT = NBLK * 2 * NB_HALF

TRACE_OPTS: dict = {}
LAST_RESULTS = None
LAST_INTERNALS: dict = {}

_prog_cache: dict = {}


def _build_program():
    key = "v10"
    if key in _prog_cache:
        return _prog_cache[key]

    f32 = mybir.dt.float32
    f16 = mybir.dt.float16
    nc = bacc.Bacc(
        "TRN2",
        target_bir_lowering=False,
        debug=False,
        enable_asserts=False,
        num_devices=NCORES,
    )
    hh = nc.dram_tensor("hh", [NPAD, 2 * H], f16, kind="ExternalInput").ap()
    hb = nc.dram_tensor("hb", [2 * H, NBT * P], f16, kind="ExternalInput").ap()
    ub = nc.dram_tensor("ub", [P, 2 * H], f16, kind="ExternalInput").ap()
    uc = nc.dram_tensor("uc", [P, 4], f16, kind="ExternalInput").ap()
    mg = nc.dram_tensor("mg", [P, TILES], f32, kind="ExternalInput").ap()
    t_out = nc.dram_tensor("t_out", [NBANK, 2 * H], f32, kind="ExternalOutput").ap()
    s_out = nc.dram_tensor("s_out", [P, 1], f32, kind="ExternalOutput").ap()

    hh_g = hh.rearrange("(p t) h -> p t h", t=TILES)
    hb_g = hb.rearrange("(c p) n -> p c n", p=P)

    HB = BLK // 2
    nA = HB - NB_HALF

    with tile.TileContext(nc) as tc:
        with (
            tc.tile_pool(name="singles", bufs=1) as singles,
            tc.tile_pool(name="blocks", bufs=5) as blocks,
            tc.tile_pool(name="small", bufs=4) as small,
            tc.tile_pool(name="scratch", bufs=3) as scratch,
            tc.tile_pool(name="psum", bufs=1, space="PSUM") as psum,
        ):
            u_sb = singles.tile([P, 2 * H], f16)
            nc.sync.dma_start(out=u_sb, in_=ub)
            uc_sb = singles.tile([P, 4], f16)
            nc.sync.dma_start(out=uc_sb, in_=uc)
            m_sb = singles.tile([P, TILES], f32)
            nc.sync.dma_start(out=m_sb, in_=mg)
            p_grid = singles.tile([P, TILES], f16)
            t_ps = []
            for j in range(NBANK):
                tpsj = psum.tile([1, 2 * H], f32, tag=f"tps{j}")
                t_ps.append(tpsj)
            c_ps = []
            for j in range(3):
                cpsj = psum.tile([P, 1], f32, tag=f"cps{j}")
                c_ps.append(cpsj)

            pending = []
            for b in range(NBLK):
                t0 = b * BLK
                buf = blocks.tile([P, BLK, 2 * H], f16)
                nc.sync.dma_start(out=buf, in_=hh_g[:, t0:t0 + BLK, :])
                tb = blocks.tile([P, 4, 2 * NB_HALF * P], f16, tag="tb")
                nc.sync.dma_start(
                    out=tb,
                    in_=hb_g[:, :, b * 2 * NB_HALF * P:(b + 1) * 2 * NB_HALF * P],
                )
                cblk = small.tile([P, BLK], f32)
                for half in range(2):
                    g0 = half * HB
                    n_act = 2 if (2 * b + half) % 2 == 0 else 3
                    for idx in range(nA):
                        g = g0 + idx
                        c = t0 + g
                        if idx < n_act:
                            scv = scratch.tile([P, 2 * H], f16, tag="dveout")
                            nc.vector.tensor_mul(scv, buf[:, g, :], u_sb)
                            sc2 = scratch.tile([P, 2 * H], f16, tag="actout")
                            nc.scalar.activation(
                                out=sc2,
                                in_=scv,
                                func=mybir.ActivationFunctionType.Identity,
                                bias=0.0,
                                scale=1.0,
                                accum_out=cblk[:, g:g + 1],
                            )
                        else:
                            sc = scratch.tile([P, 2 * H], f16, tag="sttout")
                            nc.vector.scalar_tensor_tensor(
                                out=sc,
                                in0=buf[:, g, :],
                                scalar=1.0,
                                in1=u_sb,
                                op0=mybir.AluOpType.mult,
                                op1=mybir.AluOpType.mult,
                                accum_out=cblk[:, g:g + 1],
                            )
                    for j in range(NB_HALF):
                        g = g0 + nA + j
                        c = t0 + g
                        k = (2 * half + j) * P
                        bank = c % 3
                        for ch in range(4):
                            nc.tensor.matmul(
                                c_ps[bank],
                                lhsT=tb[:, ch, k:k + P],
                                rhs=uc_sb[:, ch:ch + 1],
                                start=(ch == 0),
                                stop=(ch == 3),
                            )
                        nc.scalar.activation(
                            out=p_grid[:, c:c + 1],
                            in_=c_ps[bank],
                            func=mybir.ActivationFunctionType.Exp,
                            bias=m_sb[:, c:c + 1],
                            scale=1.0,
                        )
                    cb2 = small.tile([P, nA], f32, tag="cb2")
                    nc.vector.tensor_add(
                        cb2, cblk[:, g0:g0 + nA], m_sb[:, t0 + g0:t0 + g0 + nA]
                    )
                    nc.scalar.activation(
                        out=p_grid[:, t0 + g0:t0 + g0 + nA],
                        in_=cb2,
                        func=mybir.ActivationFunctionType.Exp,
                        bias=0.0,
                        scale=1.0,
                    )
                pending.append((t0, buf))
                if len(pending) > 1:
                    pt0, pbuf = pending.pop(0)
                    for g in range(BLK):
                        c = pt0 + g
                        nc.tensor.matmul(
                            t_ps[c % NBANK],
                            lhsT=p_grid[:, c:c + 1],
                            rhs=pbuf[:, g, :],
                            start=(c < NBANK),
                            stop=(c >= TILES - NBANK),
                        )

            for pt0, pbuf in pending:
                for g in range(BLK):
                    c = pt0 + g
                    nc.tensor.matmul(
                        t_ps[c % NBANK],
                        lhsT=p_grid[:, c:c + 1],
                        rhs=pbuf[:, g, :],
                        start=(c < NBANK),
                        stop=(c >= TILES - NBANK),
                    )

            for j in range(NBANK):
                t_sb = small.tile([1, 2 * H], f32, tag="tsb")
                nc.vector.tensor_copy(t_sb, t_ps[j])
                nc.sync.dma_start(out=t_out[j:j + 1, :], in_=t_sb)
            s_col = singles.tile([P, 1], f32)
            nc.vector.reduce_sum(out=s_col, in_=p_grid, axis=mybir.AxisListType.X)
            nc.sync.dma_start(out=s_out, in_=s_col)

    nc.compile()
    _prog_cache[key] = nc
    return nc


def _run_device(h_static, h_dynamic, u_cat, mask_bias):
    global LAST_RESULTS
    nc = _build_program()

    u16 = u_cat.astype(np.float16)
    u_bcast = np.ascontiguousarray(np.broadcast_to(u16, (P, 2 * H)))
    u_colT = np.ascontiguousarray(u16.reshape(4, P).T)

    HB = BLK // 2
    nA = HB - NB_HALF

    in_maps = []
    for c in range(NCORES):
        lo = c * NPER
        h16 = np.zeros((NPAD, 2 * H), np.float16)
        h16[:NPER, 0:H] = h_static[lo:lo + NPER]
        h16[:NPER, H:2 * H] = h_dynamic[lo:lo + NPER]
        mb = np.concatenate(
            [mask_bias[lo:lo + NPER], np.full(NPAD - NPER, NEG, np.float32)]
        )
        grid = np.ascontiguousarray(mb.reshape(P, TILES))
        hbt = np.empty((2 * H, NBT * P), np.float16)
        kk = 0
        for b in range(NBLK):
            for half in range(2):
                for j in range(NB_HALF):
                    t = b * BLK + half * HB + nA + j
                    hbt[:, kk * P:(kk + 1) * P] = h16[t::TILES, :].T
                    kk += 1
        in_maps.append({"hh": h16, "hb": np.ascontiguousarray(hbt),
                        "ub": u_bcast, "uc": u_colT, "mg": grid})

    res = bass_utils.run_bass_kernel_spmd(
        nc, in_maps, core_ids=list(range(NCORES)), **TRACE_OPTS
    )
    LAST_RESULTS = res

    t = np.zeros(2 * H, np.float64)
    s = 0.0
    for c in range(NCORES):
        t += res.results[c]["t_out"].astype(np.float64).sum(axis=0)
        s += float(res.results[c]["s_out"].astype(np.float64).sum())
    return t, s


def kernel(
    h_dynamic,
    h_static,
    W_static_kvl,
    W_dyn_kvl,
    W_q,
    W1,
    b1,
    W2,
    b2,
    valid_mask,
    current_node,
):
    h_dynamic = np.asarray(h_dynamic, np.float32)
    h_static = np.asarray(h_static, np.float32)
    W_static_kvl = np.asarray(W_static_kvl, np.float32)
    W_dyn_kvl = np.asarray(W_dyn_kvl, np.float32)
    W_q = np.asarray(W_q, np.float32)
    W1 = np.asarray(W1, np.float32)
    b1 = np.asarray(b1, np.float32)
    W2 = np.asarray(W2, np.float32)
    b2 = np.asarray(b2, np.float32)
    valid = np.asarray(valid_mask).astype(bool)
    cur = int(current_node)

    scale = 1.0 / math.sqrt(H)

    h_cur = (h_static[cur].astype(np.float64) + h_dynamic[cur].astype(np.float64))
    q = h_cur @ W_q.astype(np.float64)
    u_s = (W_static_kvl[:, 0:H].astype(np.float64) @ q) * scale
    u_d = (W_dyn_kvl[:, 0:H].astype(np.float64) @ q) * scale
    u_cat = np.concatenate([u_s, u_d]).astype(np.float32)

    mask_bias = np.where(valid, np.float32(-SHIFT), NEG).astype(np.float32)

    t, s = _run_device(h_static, h_dynamic, u_cat, mask_bias)

    W_vs = W_static_kvl[:, H:2 * H].astype(np.float64)
    W_vd = W_dyn_kvl[:, H:2 * H].astype(np.float64)
    context = (t[:H] @ W_vs + t[H:] @ W_vd) / s

    fuse = np.concatenate([h_cur, context])
    hidden = np.maximum(fuse @ W1.astype(np.float64) + b1.astype(np.float64), 0.0)
    logit = float(hidden @ W2.astype(np.float64)[:, 0] + float(b2[0]))

    logits_all = np.where(valid, np.float32(logit), NEG).astype(np.float32)

    LAST_INTERNALS.update(
        dict(u_cat=u_cat, t=t, s=s, context=context, logit=logit)
    )

    import jax
    import jax.numpy as jnp

    cpu = jax.devices("cpu")[0]
    with jax.default_device(cpu):
        logits_j = jnp.asarray(logits_all)
        choice = jax.random.categorical(jax.random.key(1), logits_j)
        log_probs = jax.nn.log_softmax(logits_j)
        log_prob = log_probs[choice]
        choice_np = np.asarray(choice)
        log_prob_np = np.asarray(log_prob)

    return (choice_np, log_prob_np)
